# revision 1
# baseline (speedup 1.0000x reference)
"""Trainium2 Bass kernel for the ETD1 ODE block (nn_ODEblockW_28922309771809).

Math (mirrors the jax reference, but solve-free):
  s    = 0.05 * sigmoid(alpha)                       # row scales (0.5*dt)
  X    = dt*A = diag(s) @ (adj - I)                  # [2048,2048], ||X|| ~ 0.073
  m1_L = e^X     via degree-8 Taylor, Paterson-Stockmeyer with Y = X^3
  m2   = A^{-1}(e^X - I) = dt*phi1(X),  phi1 = sum_k X^k/(k+1)!   (degree-8 PS)
  B    = (w*clip(d,0,1)) @ w.T - I  (symmetric);  Xr = dt*B;  m1_R = e^{Xr}
  F    = m2 @ x0
  z    = IC after 9 steps of IC <- m1_L @ IC @ m1_R + F   (N_STEPS = int(1.0//0.1) == 9)

Distribution over 8 cores (transposed-column-local formulation):
  The node dim (2048) is sharded 256 rows/core; every local tensor is held as
  the transposed column block [2048|1024, 256], so each big matmul is
     out_colT[m] = sum_k  matmul(lhsT = Full[kblk, mblk] from DRAM, rhs = colT[kblk])
  Full matrices are assembled by AllGather of row blocks (PE-transpose of the
  local column block first). The feature dim (1024) is sharded 128/core.

  AllGathered tensors use a TILED layout: each rank's contribution is a
  sequence of [128,128] tiles (m-major), so the per-m lhsT slab loads read
  8-16 contiguous 32-64KB blocks instead of 256B-strided rows. Node-dim
  gathers are split into two pipelined half-gathers (half j carries k-chunks
  k%2==j); consuming matmuls run even k-chunks first so they start as soon as
  the first half lands.

Precision: series matmuls in bf16 (bf16 error only enters quadratic+ Taylor
terms of e^X; the I and X terms are exact fp32 elementwise), recurrence /
forcing / R-side matmuls in float32r. Measured ~9.5e-4 frob rel err vs the
fp32 reference, which itself carries ~1.7e-4 fp32 rounding noise vs fp64.
"""

import math
from contextlib import ExitStack

import numpy as np

import concourse.bass as bass
import concourse.mybir as mybir
import concourse.tile as tile
from concourse import bacc
from concourse.bass_utils import run_bass_kernel_spmd
from concourse.masks import make_identity

F32 = mybir.dt.float32
F32R = mybir.dt.float32r
BF16 = mybir.dt.bfloat16
AL = mybir.AluOpType

N_CORES = 8
P = 128
N = 2048          # nodes
D = 1024          # features
RB = 256          # node row-block per core
FB = 256          # node col-block width (L side)
FBR = 128         # feature block width (R side, true 8-way shard)
NKC = N // P      # 16
DKC = D // P      # 8
RJ = RB // P      # 2
NSTEPS = 9        # int(1.0 // 0.1) == 9

EC = [1.0 / math.factorial(k) for k in range(9)]        # e^X coeffs
PC = [0.1 / math.factorial(k + 1) for k in range(9)]    # dt*phi1(X) coeffs

LGROUP = [list(range(N_CORES))]


def build_nc():
    nc = bacc.Bacc("TRN2", target_bir_lowering=False, debug=False,
                   num_devices=N_CORES)

    # ---- I/O (per-core shards fed host-side; same NEFF on all cores) ----
    adj_rows = nc.dram_tensor("adj_rows", [RB, N], F32, kind="ExternalInput")
    eye_rows = nc.dram_tensor("eye_rows", [RB, N], F32, kind="ExternalInput")
    eye_colT = nc.dram_tensor("eye_colT", [N, RB], F32, kind="ExternalInput")
    alpha_blk = nc.dram_tensor("alpha_blk", [RB], F32, kind="ExternalInput")
    x_full = nc.dram_tensor("x_full", [N, D], F32, kind="ExternalInput")
    x0_full = nc.dram_tensor("x0_full", [N, D], F32, kind="ExternalInput")
    w_cols = nc.dram_tensor("w_cols", [D, FBR], F32, kind="ExternalInput")
    w_rows = nc.dram_tensor("w_rows", [FBR, D], F32, kind="ExternalInput")
    eye_feat = nc.dram_tensor("eye_feat", [D, FBR], F32, kind="ExternalInput")
    d_full = nc.dram_tensor("d_full", [D], F32, kind="ExternalInput")
    z_loc = nc.dram_tensor("z_loc", [RB, D], F32, kind="ExternalOutput")

    with tile.TileContext(nc) as tc, ExitStack() as top:
        const = top.enter_context(tc.tile_pool(name="const", bufs=1))
        dram = top.enter_context(tc.tile_pool(name="dram", bufs=1, space="DRAM"))
        psum = top.enter_context(tc.tile_pool(name="psum", bufs=2, space="PSUM"))
        slabp = top.enter_context(tc.tile_pool(name="slabp", bufs=1))
        scrp = top.enter_context(tc.tile_pool(name="scrp", bufs=1))
        lser = top.enter_context(tc.tile_pool(name="lser", bufs=1))
        lout = top.enter_context(tc.tile_pool(name="lout", bufs=1))

        ident = const.tile([P, P], F32)
        make_identity(nc, ident)
        ident_b = const.tile([P, P], BF16)
        nc.vector.tensor_copy(ident_b[:], ident[:])

        def pe_t(dst_slice, src_slice, bf=False):
            """dst[128,128] = src[128,128].T via PE transpose."""
            if src_slice.dtype == F32R:
                src_slice = src_slice.bitcast(F32)
            ps = psum.tile([P, P], BF16 if bf else F32, tag="tr", bufs=4, name="ps_tr")
            nc.tensor.transpose(ps[:], src_slice, ident_b[:] if bf else ident[:])
            nc.vector.tensor_copy(dst_slice, ps[:])

        def combo(dst_slice, eye_m, xt_m, x2t_m, c0, c1, c2):
            """dst = c0*I + c1*X + c2*X2 for one [128,w] chunk."""
            if xt_m.dtype == F32R:
                xt_m = xt_m.bitcast(F32)
            if x2t_m.dtype == F32R:
                x2t_m = x2t_m.bitcast(F32)
            w = xt_m.shape[-1]
            st = scrp.tile([P, FB], F32, tag="combo", bufs=3, name="combo_scr")
            s = st[:, :w]
            nc.vector.tensor_scalar_mul(s, xt_m, c1)
            nc.vector.scalar_tensor_tensor(s, x2t_m, c2, s, AL.mult, AL.add)
            nc.vector.scalar_tensor_tensor(dst_slice, eye_m, c0, s, AL.mult, AL.add)

        def load_eye(dram_t, m, w=FB):
            t = scrp.tile([P, FB], F32, tag="eye", bufs=2, name="eye_chunk")
            nc.sync.dma_start(t[:, :w], dram_t[m * P:(m + 1) * P, :])
            return t[:, :w]

        def _bc(src_ap, dt):
            if dt == F32R and src_ap.dtype == F32:
                return src_ap.bitcast(F32R)
            return src_ap

        # ---- tiled-gather helpers -------------------------------------
        # A gathered tensor is [ranks * tiles * P, P]: rank c's contribution
        # is `tiles` contiguous [128,128] tiles (tile t = cols t*128 of the
        # rank's [128, tiles*128] row block).
        def put_tiles(ccin, row_sb, tiles):
            """DMA row block row_sb [128, tiles*128] into tiled ccin."""
            for t in range(tiles):
                nc.sync.dma_start(ccin[t * P:(t + 1) * P, :],
                                  row_sb[:, t * P:(t + 1) * P])

        def tiled_src(g, m, jpr, tiles, dt):
            """AP over gathered g: [128, ranks, jpr, 128] = tile m of every
            rank's jpr row-chunks. Contribution tile order: j-major, m-minor."""
            a = _bc(g[:], dt).rearrange("(c j t p) n -> p c j t n", c=N_CORES,
                                        j=jpr, t=tiles, p=P)
            return a[:, :, :, m, :]

        def gather_tiled(produce, jpr, tiles, dt, name):
            """Single AllGather with tiled contribution: jpr row-chunks of
            `tiles` [128,128] tiles each. produce(ccin) fills it."""
            ccin = dram.tile([jpr * tiles * P, P], dt, tag=f"ccin_{name}",
                             name=f"ccin_{name}")
            full = dram.tile([N_CORES * jpr * tiles * P, P], dt,
                             addr_space="Shared", name=f"full_{name}")
            produce(ccin)
            nc.gpsimd.collective_compute(
                "AllGather", AL.bypass, replica_groups=LGROUP,
                ins=[ccin.opt()], outs=[full.opt()])
            return full

        def mm_pass(rhs_tiles, n_k, n_m, evict, dt, tag, nb=FB,
                    g=None, jpr=1, plain=None, tiles=None):
            """For each output chunk m: psums[i] = sum_k lhsT[k,m].T @ rhs[i][k].

            lhsT source: either `plain` (a [n_k*P, n_m*P] DRAM AP, k-chunk k at
            rows k*128) or `g` (a tiled-gathered tensor where k-chunk k lives
            as rank k//jpr, row-chunk k%jpr). For dt == F32R the rhs tiles
            must already be float32r-dtyped."""
            tiles_ = n_m if tiles is None else tiles
            for m in range(n_m):
                if g is not None:
                    sl = slabp.tile([P, N_CORES, jpr, P], dt, tag=tag,
                                    bufs=2, name=f"slab_{tag}")
                    nc.sync.dma_start(sl[:], tiled_src(g, m, jpr, tiles_, dt))
                    lt = lambda k: sl[:, k // jpr, k % jpr, :]
                else:
                    sl = slabp.tile([P, n_k, P], dt, tag=tag, bufs=2,
                                    name=f"slab_{tag}")
                    src = _bc(plain[:, m * P:(m + 1) * P], dt)
                    nc.sync.dma_start(sl[:], src.rearrange("(k p) n -> p k n", p=P))
                    lt = lambda k: sl[:, k, :]
                pss = [psum.tile([P, nb], F32, tag=f"mm{i}", bufs=2,
                                 name=f"ps_mm{i}") for i in range(len(rhs_tiles))]
                for k in range(n_k):
                    for ps, rhs in zip(pss, rhs_tiles):
                        nc.tensor.matmul(ps[:], lt(k), rhs[:, k, :],
                                         start=(k == 0), stop=(k == n_k - 1))
                evict(m, pss)

        # =========================================================
        # Prep scales
        # =========================================================
        s_sb = const.tile([P, RJ], F32)
        nc.sync.dma_start(s_sb[:], alpha_blk.ap().rearrange("(j p) -> p j", p=P))
        nc.scalar.activation(s_sb[:], s_sb[:], mybir.ActivationFunctionType.Sigmoid)
        nc.vector.tensor_scalar_mul(s_sb[:], s_sb[:], 0.05)

        d_sb = const.tile([P, DKC], F32)
        nc.sync.dma_start(d_sb[:], d_full.ap().rearrange("(q p) -> p q", p=P))
        nc.vector.tensor_scalar(d_sb[:], d_sb[:], 0.0, 1.0, AL.max, AL.min)

        xt = lser.tile([P, NKC, FB], F32)     # X^T col block, fp32
        x2t = lser.tile([P, NKC, FB], F32)    # (X^2)^T col block, fp32
        et = lout.tile([P, NKC, FB], F32R)    # m1_L^T col block
        m2t = lout.tile([P, NKC, FB], F32R)   # m2^T col block

        # =========================================================
        # Emission order interleaves the R-side (feature dim) chain between
        # the L-side passes: engine queues are in-order, so each R compute
        # segment is emitted one L-pass after the gather it depends on —
        # its semaphore wait is satisfied by the time the PE reaches it.
        # =========================================================
        pa_st, pr_st = ExitStack(), ExitStack()
        pr = pr_st.enter_context(tc.tile_pool(name="ph_r", bufs=1))
        pa = pa_st.enter_context(tc.tile_pool(name="ph_a", bufs=1))

        # --- R prep: w^T row block -> AllGather (earliest collective) ---
        wt_rowblk = pr.tile([P, D], F32)
        for k in range(DKC):
            wc_sb = pr.tile([P, FBR], F32, tag="w_in", bufs=2, name="wc_sb")
            nc.sync.dma_start(wc_sb[:], w_cols[k * P:(k + 1) * P, :])
            pe_t(wt_rowblk[:, k * P:(k + 1) * P], wc_sb[:])
        wt_g = gather_tiled(lambda ccin: put_tiles(ccin, wt_rowblk[:], DKC),
                            1, DKC, F32, "wt")

        # V = diag(clip(d)) @ w^T[:, Fblk]   [1024, 128]
        vr = pr.tile([P, DKC, FBR], F32R)
        wr_sb = pr.tile([P, D], F32, name="wr_sb")
        nc.sync.dma_start(wr_sb[:], w_rows[:])
        for k in range(DKC):
            pe_t(vr[:, k, :], wr_sb[:, k * P:(k + 1) * P])
        for k in range(DKC):
            nc.vector.tensor_scalar_mul(vr[:, k, :], vr[:, k, :].bitcast(F32),
                                        d_sb[:, k:k + 1])

        # --- Phase A: build X row block, AllGather X (bf16), transpose ---
        xrow = pa.tile([P, RJ, N], F32)
        xrow_b = pa.tile([P, RJ, N], BF16)
        ccin_x = dram.tile([RJ * NKC * P, P], BF16, name="ccin_x")
        for j in range(RJ):
            adj_sb = pa.tile([P, N], F32, tag="a_in", bufs=2, name="adj_sb")
            eyer_sb = pa.tile([P, N], F32, tag="a_in", bufs=2, name="eyer_sb")
            nc.sync.dma_start(adj_sb[:], adj_rows[j * P:(j + 1) * P, :])
            nc.sync.dma_start(eyer_sb[:], eye_rows[j * P:(j + 1) * P, :])
            nc.vector.tensor_sub(adj_sb[:], adj_sb[:], eyer_sb[:])
            nc.vector.tensor_scalar_mul(xrow[:, j, :], adj_sb[:], s_sb[:, j:j + 1])
            nc.vector.tensor_copy(xrow_b[:, j, :], xrow[:, j, :])
            put_tiles(ccin_x[j * NKC * P:(j + 1) * NKC * P, :], xrow_b[:, j, :], NKC)
        xfull_g = dram.tile([N_CORES * RJ * NKC * P, P], BF16,
                            addr_space="Shared", name="full_x")
        nc.gpsimd.collective_compute(
            "AllGather", AL.bypass, replica_groups=LGROUP,
            ins=[ccin_x.opt()], outs=[xfull_g.opt()])

        for k in range(NKC):
            for j in range(RJ):
                pe_t(xt[:, k, j * P:(j + 1) * P], xrow[:, j, k * P:(k + 1) * P])
        pa_st.close()

        # --- R: w_mat col block -> Xr = 0.1*(w_mat - I); gather Xr ---
        xr_col = pr.tile([P, DKC, FBR], F32R)

        def ev_wmat(m, pss):
            eyef = load_eye(eye_feat, m, FBR)
            nc.vector.tensor_sub(xr_col[:, m, :], pss[0][:], eyef)
            nc.vector.tensor_scalar_mul(xr_col[:, m, :],
                                        xr_col[:, m, :].bitcast(F32), 0.1)
        mm_pass([vr], DKC, DKC, ev_wmat, F32R, "fslab", nb=FBR,
                g=wt_g, tiles=DKC)

        def gather_sym(col_tile, name):
            """Symmetric [D,D] matrix: transpose col block -> row block -> AG."""
            rowblk = pr.tile([P, D], F32, tag="r_rowblk", bufs=2,
                             name=f"rowblk_{name}")
            for k in range(DKC):
                pe_t(rowblk[:, k * P:(k + 1) * P], col_tile[:, k, :])
            return gather_tiled(lambda ccin: put_tiles(ccin, rowblk[:], DKC),
                                1, DKC, F32, name)

        xr_g = gather_sym(xr_col, "xr")

        # --- Phase C1: X^2 (bf16) ---
        pc_st = ExitStack()
        pc_ = pc_st.enter_context(tc.tile_pool(name="ph_c", bufs=1))
        xt_b = pc_.tile([P, NKC, FB], BF16)
        nc.vector.tensor_copy(xt_b[:], xt[:])
        x2t_b = pc_.tile([P, NKC, FB], BF16)

        def ev_x2(m, pss):
            nc.vector.tensor_copy(x2t[:, m, :], pss[0][:])
            nc.vector.tensor_copy(x2t_b[:, m, :], pss[0][:])
        mm_pass([xt_b], NKC, NKC, ev_x2, BF16, "xslab", g=xfull_g, jpr=RJ)

        # --- R: Xr^2, Xr^3 (gathers hidden under the X^2 pass) ---
        xr2_col = pr.tile([P, DKC, FBR], F32R)
        mm_pass([xr_col], DKC, DKC,
                lambda m, pss: nc.vector.tensor_copy(xr2_col[:, m, :], pss[0][:]),
                F32R, "fslab", nb=FBR, g=xr_g, tiles=DKC)
        xr3_col = pr.tile([P, DKC, FBR], F32)
        mm_pass([xr2_col], DKC, DKC,
                lambda m, pss: nc.vector.tensor_copy(xr3_col[:, m, :], pss[0][:]),
                F32R, "fslab", nb=FBR, g=xr_g, tiles=DKC)
        xr3_g = gather_sym(xr3_col, "xr3")

        # --- Phase C2: X^3 (bf16) ---
        x3t_b = pc_.tile([P, NKC, FB], BF16)
        mm_pass([x2t_b], NKC, NKC,
                lambda m, pss: nc.vector.tensor_copy(x3t_b[:, m, :], pss[0][:]),
                BF16, "xslab", g=xfull_g, jpr=RJ)

        x3row_b = pc_.tile([P, RJ, N], BF16)
        ccin_x3 = dram.tile([RJ * NKC * P, P], BF16, name="ccin_x3")
        for j in range(RJ):
            for k in range(NKC):
                pe_t(x3row_b[:, j, k * P:(k + 1) * P],
                     x3t_b[:, k, j * P:(j + 1) * P], bf=True)
            put_tiles(ccin_x3[j * NKC * P:(j + 1) * NKC * P, :],
                      x3row_b[:, j, :], NKC)
        x3full_g = dram.tile([N_CORES * RJ * NKC * P, P], BF16,
                             addr_space="Shared", name="full_x3")
        nc.gpsimd.collective_compute(
            "AllGather", AL.bypass, replica_groups=LGROUP,
            ins=[ccin_x3.opt()], outs=[x3full_g.opt()])

        pc_st.close()

        # --- R: T_R = B1r + Y*B2r ; m1_R = B0r + Y*T_R  (xr3 gather done
        #     during the X^3 pass) ---
        b2r = pr.tile([P, DKC, FBR], F32R)
        for m in range(DKC):
            eyef = load_eye(eye_feat, m, FBR)
            combo(b2r[:, m, :], eyef, xr_col[:, m, :], xr2_col[:, m, :],
                  EC[6], EC[7], EC[8])
        tr_col = pr.tile([P, DKC, FBR], F32R)

        def ev_tr(m, pss):
            eyef = load_eye(eye_feat, m, FBR)
            b1t = scrp.tile([P, FB], F32, tag="combo", bufs=3, name="b1_scr")
            b1 = b1t[:, :FBR]
            combo(b1, eyef, xr_col[:, m, :], xr2_col[:, m, :],
                  EC[3], EC[4], EC[5])
            nc.vector.tensor_add(tr_col[:, m, :], pss[0][:], b1)
        mm_pass([b2r], DKC, DKC, ev_tr, F32R, "fslab", nb=FBR,
                g=xr3_g, tiles=DKC)

        m1r_col = pr.tile([P, DKC, FBR], F32)

        def ev_m1r(m, pss):
            eyef = load_eye(eye_feat, m, FBR)
            b0t = scrp.tile([P, FB], F32, tag="combo", bufs=3, name="b0_scr")
            b0 = b0t[:, :FBR]
            combo(b0, eyef, xr_col[:, m, :], xr2_col[:, m, :],
                  EC[0], EC[1], EC[2])
            nc.vector.tensor_add(m1r_col[:, m, :], pss[0][:], b0)
        mm_pass([tr_col], DKC, DKC, ev_m1r, F32R, "fslab", nb=FBR,
                g=xr3_g, tiles=DKC)

        m1r_g = gather_sym(m1r_col, "m1r")
        pr_st.close()

        # --- Phase D: T/S then E/P Horner steps (bf16) ---
        pd_st = ExitStack()
        pd = pd_st.enter_context(tc.tile_pool(name="ph_d", bufs=1))
        b2e_b = pd.tile([P, NKC, FB], BF16)
        c2p_b = pd.tile([P, NKC, FB], BF16)
        for m in range(NKC):
            eyet = load_eye(eye_colT, m)
            combo(b2e_b[:, m, :], eyet, xt[:, m, :], x2t[:, m, :],
                  EC[6], EC[7], EC[8])
            combo(c2p_b[:, m, :], eyet, xt[:, m, :], x2t[:, m, :],
                  PC[6], PC[7], PC[8])

        tt_b = pd.tile([P, NKC, FB], BF16)
        st_b = pd.tile([P, NKC, FB], BF16)

        def ev_ts(m, pss):
            eyet = load_eye(eye_colT, m)
            b1 = scrp.tile([P, FB], F32, tag="combo", bufs=3, name="ts_scr")
            combo(b1[:], eyet, xt[:, m, :], x2t[:, m, :], EC[3], EC[4], EC[5])
            nc.vector.tensor_add(tt_b[:, m, :], pss[0][:], b1[:])
            combo(b1[:], eyet, xt[:, m, :], x2t[:, m, :], PC[3], PC[4], PC[5])
            nc.vector.tensor_add(st_b[:, m, :], pss[1][:], b1[:])
        mm_pass([b2e_b, c2p_b], NKC, NKC, ev_ts, BF16, "xslab",
                g=x3full_g, jpr=RJ)

        def ev_ep(m, pss):
            eyet = load_eye(eye_colT, m)
            b0 = scrp.tile([P, FB], F32, tag="combo", bufs=3, name="ep_scr")
            combo(b0[:], eyet, xt[:, m, :], x2t[:, m, :], EC[0], EC[1], EC[2])
            nc.vector.tensor_add(et[:, m, :], pss[0][:], b0[:])
            combo(b0[:], eyet, xt[:, m, :], x2t[:, m, :], PC[0], PC[1], PC[2])
            nc.vector.tensor_add(m2t[:, m, :], pss[1][:], b0[:])
        mm_pass([tt_b, st_b], NKC, NKC, ev_ep, BF16, "xslab",
                g=x3full_g, jpr=RJ)

        pd_st.close()

        # --- Phase E: forcing + 9-step recurrence (fp32r) ---
        pe = top.enter_context(tc.tile_pool(name="ph_e", bufs=1))
        m1r_sb = pe.tile([P, DKC, DKC, P], F32R)
        nc.sync.dma_start(
            m1r_sb[:],
            m1r_g[:].bitcast(F32R).rearrange("(c t p) n -> p c t n",
                                             c=N_CORES, t=DKC, p=P))

        ft = pe.tile([P, DKC, FB], F32)
        mm_pass([m2t], NKC, DKC,
                lambda m, pss: nc.vector.tensor_copy(ft[:, m, :], pss[0][:]),
                F32R, "icslab0", plain=x0_full[:])

        ic_g = None
        for t in range(NSTEPS):
            # V = (m1_L @ IC)^T col block = IC^T-contract with m1_L^T col
            v = pe.tile([P, DKC, FB], F32R, tag="v_step", bufs=2, name="v")
            if t == 0:
                mm_pass([et], NKC, DKC,
                        lambda m, pss: nc.vector.tensor_copy(v[:, m, :], pss[0][:]),
                        F32R, "icslab0", plain=x_full[:])
            else:
                mm_pass([et], NKC, DKC,
                        lambda m, pss: nc.vector.tensor_copy(v[:, m, :], pss[0][:]),
                        F32R, "icslab", g=ic_g, jpr=RJ, tiles=DKC)
            # IC_new^T col = m1_R-contract with V + F^T
            icnt = pe.tile([P, DKC, FB], F32, tag="icnt_step", bufs=2, name="icnt")
            for m in range(DKC):
                ps = psum.tile([P, FB], F32, tag="mm0", bufs=2, name="ps_rec")
                for k in range(DKC):
                    nc.tensor.matmul(
                        ps[:], m1r_sb[:, k, m, :], v[:, k, :],
                        start=(k == 0), stop=(k == DKC - 1))
                nc.vector.tensor_add(icnt[:, m, :], ps[:], ft[:, m, :])
            # transpose to row block; DMA tiles out as they complete
            icrow = pe.tile([P, RJ, D], F32, tag="icrow_step", bufs=2, name="icrow")
            if t < NSTEPS - 1:
                ccin_ic = dram.tile([RJ * DKC * P, P], F32, tag="ccin_ic",
                                    name=f"ccin_ic{t}")
                for j in range(RJ):
                    for m in range(DKC):
                        pe_t(icrow[:, j, m * P:(m + 1) * P],
                             icnt[:, m, j * P:(j + 1) * P])
                        nc.sync.dma_start(
                            ccin_ic[(j * DKC + m) * P:(j * DKC + m + 1) * P, :],
                            icrow[:, j, m * P:(m + 1) * P])
                ic_g = dram.tile([N_CORES * RJ * DKC * P, P], F32,
                                 addr_space="Shared", name=f"full_ic{t}")
                nc.gpsimd.collective_compute(
                    "AllGather", AL.bypass, replica_groups=LGROUP,
                    ins=[ccin_ic.opt()], outs=[ic_g.opt()])
            else:
                for j in range(RJ):
                    for m in range(DKC):
                        pe_t(icrow[:, j, m * P:(m + 1) * P],
                             icnt[:, m, j * P:(j + 1) * P])
                    nc.sync.dma_start(z_loc[j * P:(j + 1) * P, :], icrow[:, j, :])

    nc.compile()
    return nc


_NC_CACHE = []


def _get_nc():
    if not _NC_CACHE:
        _NC_CACHE.append(build_nc())
    return _NC_CACHE[0]


def make_in_maps(inputs):
    x = np.ascontiguousarray(np.asarray(inputs["x"], dtype=np.float32))
    x0 = np.ascontiguousarray(np.asarray(inputs["x0"], dtype=np.float32))
    adj = np.ascontiguousarray(np.asarray(inputs["adj"], dtype=np.float32))
    alpha = np.ascontiguousarray(np.asarray(inputs["alpha_train"], dtype=np.float32))
    w = np.ascontiguousarray(np.asarray(inputs["w"], dtype=np.float32))
    d = np.ascontiguousarray(np.asarray(inputs["d"], dtype=np.float32))

    eye_n = np.eye(N, dtype=np.float32)
    eye_d = np.eye(D, dtype=np.float32)

    in_maps = []
    for c in range(N_CORES):
        r0 = c * RB
        f0 = c * FBR
        in_maps.append({
            "adj_rows": np.ascontiguousarray(adj[r0:r0 + RB, :]),
            "eye_rows": np.ascontiguousarray(eye_n[r0:r0 + RB, :]),
            "eye_colT": np.ascontiguousarray(eye_n[:, r0:r0 + RB]),
            "alpha_blk": np.ascontiguousarray(alpha[r0:r0 + RB]),
            "x_full": x,
            "x0_full": x0,
            "w_cols": np.ascontiguousarray(w[:, f0:f0 + FBR]),
            "w_rows": np.ascontiguousarray(w[f0:f0 + FBR, :]),
            "eye_feat": np.ascontiguousarray(eye_d[:, f0:f0 + FBR]),
            "d_full": d,
        })
    return in_maps


def kernel(**inputs) -> np.ndarray:
    nc = _get_nc()
    in_maps = make_in_maps(inputs)
    res = run_bass_kernel_spmd(nc, in_maps, core_ids=list(range(N_CORES)))
    z = np.concatenate([res.results[c]["z_loc"] for c in range(N_CORES)], axis=0)
    return np.ascontiguousarray(z.astype(np.float32))


if __name__ == "__main__":
    rng = np.random.default_rng(0)
    ins = {
        "x": rng.standard_normal((N, D)).astype(np.float32),
        "x0": rng.standard_normal((N, D)).astype(np.float32),
        "adj": (rng.random((N, N)) / N).astype(np.float32),
        "alpha_train": rng.standard_normal((N,)).astype(np.float32),
        "w": (np.eye(D) + 0.02 * rng.standard_normal((D, D))).astype(np.float32),
        "d": rng.random((D,)).astype(np.float32),
    }
    out = kernel(**ins)
    print("kernel output:", out.shape, out.dtype, float(np.linalg.norm(out)))



# revision 21
# speedup vs baseline: 2.2640x; 2.2640x over previous
"""Trainium2 Bass kernel for the ETD1 ODE block (nn_ODEblockW_28922309771809).

Math (mirrors the jax reference; 9 steps of IC <- L IC R + F regrouped as
3 strides of 3):
  X  = dt*A = diag(0.05*sigmoid(alpha)) @ (adj - I)    ||X||_2 ~ 0.05
  Xr = dt*B = 0.1*((w*clip(d,0,1)) @ w.T - I)          ||Xr||_2 ~ 0.18
  L^t and L^t@m2 are elementwise combos of powers of the SAME X, so the
  3-step forcing F3 = F + L F R + L^2 F R^2 is built collective-free from
  the replicated x0:
    P_t = G_t @ x0,  G_t = L^t m2 = 0.1*(I + c1 X + c2 X^2 + c3 X^3),
          c_k(t) = ((t+1)^{k+1} - t^{k+1})/(k+1)!
    F3  = P0 + (P1 + P2@R)@R,   Z@R = Z + Z@ER   (delta form)
  EL3 = e^{3X} - I (deg 3), ER = e^{Xr} - I (deg 4), ER3 = e^{3Xr} - I
  (deg 6; all R powers need only the Xr gather as lhsT).
  IC_{t+3} = S + S@ER3 + F3,  S = IC + EL3@IC  -- 3 steps; only the two
  intermediate ICs are all-gathered (bf16); IC_0 = x is replicated.

Distribution over 8 cores: node dim sharded 256 rows/core (row blocks in
the recurrence, transposed col blocks as series lhsT), feature dim
sharded 128/core for the R-side series. All series/recurrence matmuls
bf16 (fp32 PSUM accumulation); the delta form keeps bf16 rounding scaled
by ||e^. - I|| << 1. wmat runs in fp32r. Numpy bit-sim of this exact
scheme: 9.4e-4 frob rel err vs the fp64 reference.
"""

import math
from contextlib import ExitStack

import numpy as np

import concourse.bass as bass
import concourse.mybir as mybir
import concourse.tile as tile
from concourse import bacc
from concourse.bass_utils import run_bass_kernel_spmd
from concourse.masks import make_identity

F32 = mybir.dt.float32
F32R = mybir.dt.float32r
BF16 = mybir.dt.bfloat16
AL = mybir.AluOpType
AF = mybir.ActivationFunctionType

N_CORES = 8
P = 128
N = 2048          # nodes
D = 1024          # features
RB = 256          # node rows per core
FBR = 128         # feature cols per core
NKC = N // P      # 16
DKC = D // P      # 8
RJ = RB // P      # 2
FH = 512          # free-dim half for row-form matmuls (1 PSUM bank)

LGROUP = [list(range(N_CORES))]


def _gc(t, k):
    return 0.1 * ((t + 1) ** (k + 1) - t ** (k + 1)) / math.factorial(k + 1)


GC = [[_gc(t, k) for k in range(1, 4)] for t in range(3)]   # G_t delta coeffs
EL3C = [3.0, 4.5, 4.5]                                      # e^{3X}-I deg3
ER3C = [3.0, 4.5, 4.5, 3.375, 2.025, 1.0125]                # e^{3Xr}-I deg6


def build_nc():
    nc = bacc.Bacc("TRN2", target_bir_lowering=False, debug=False,
                   num_devices=N_CORES)

    # adj_rows is host-side pre-subtracted: adj[rows] - I[rows]
    adj_rows = nc.dram_tensor("adj_rows", [RB, N], F32, kind="ExternalInput")
    alpha_blk = nc.dram_tensor("alpha_blk", [RB], F32, kind="ExternalInput")
    x_full = nc.dram_tensor("x_full", [N, D], F32, kind="ExternalInput")
    x0_full = nc.dram_tensor("x0_full", [N, D], F32, kind="ExternalInput")
    x_locd = nc.dram_tensor("x_loc", [RB, D], F32, kind="ExternalInput")
    x0_locd = nc.dram_tensor("x0_loc", [RB, D], F32, kind="ExternalInput")
    wT_full = nc.dram_tensor("wT_full", [D, D], F32, kind="ExternalInput")
    wt_cols = nc.dram_tensor("wt_cols", [D, FBR], F32, kind="ExternalInput")
    eye_feat = nc.dram_tensor("eye_feat", [D, FBR], F32, kind="ExternalInput")
    d_full = nc.dram_tensor("d_full", [D], F32, kind="ExternalInput")
    z_loc = nc.dram_tensor("z_loc", [RB, D], F32, kind="ExternalOutput")

    with tile.TileContext(nc) as tc, ExitStack() as top:
        const = top.enter_context(tc.tile_pool(name="const", bufs=1))
        dram = top.enter_context(tc.tile_pool(name="dram", bufs=1, space="DRAM"))
        # PSUM budget (8 banks): trf(1)+trb(1) persistent; mmL(2)+mmR(2) in
        # psf (front, closed before P passes); f0..f5 (6) in psr after.
        psum = top.enter_context(tc.tile_pool(name="psum", bufs=2, space="PSUM"))
        pf_st = ExitStack()
        psf = pf_st.enter_context(tc.tile_pool(name="psumf", bufs=2,
                                               space="PSUM"))
        slabp = top.enter_context(tc.tile_pool(name="slabp", bufs=1))
        scrp = top.enter_context(tc.tile_pool(name="scrp", bufs=1))
        lser = top.enter_context(tc.tile_pool(name="lser", bufs=1))
        rser = top.enter_context(tc.tile_pool(name="rser", bufs=1))
        recp = top.enter_context(tc.tile_pool(name="recp", bufs=1))

        ident = const.tile([P, P], F32)
        make_identity(nc, ident)
        ident_b = const.tile([P, P], BF16)
        nc.vector.tensor_copy(ident_b[:], ident[:])

        def pe_t(dst_slice, src_slice):
            """dst[128,128] = src[128,128].T via PE transpose (the PSUM->SBUF
            copy converts dtype if dst differs)."""
            if src_slice.dtype == F32R:
                src_slice = src_slice.bitcast(F32)
            bf = src_slice.dtype == BF16
            ps = psum.tile([P, P], BF16 if bf else F32,
                           tag="trb" if bf else "trf", bufs=1, name="ps_tr")
            nc.tensor.transpose(ps[:], src_slice, ident_b[:] if bf else ident[:])
            nc.vector.tensor_copy(dst_slice, ps[:])

        def put_tiles(ccin, row_sb, tiles):
            for t in range(tiles):
                nc.sync.dma_start(ccin[t * P:(t + 1) * P, :],
                                  row_sb[:, t * P:(t + 1) * P])

        def tiled_src(g, m, jpr, tiles):
            a = g[:].rearrange("(c j t p) n -> p c j t n", c=N_CORES,
                               j=jpr, t=tiles, p=P)
            return a[:, :, :, m, :]

        # =========================================================
        # Prep scales + local x/x0 rows
        # =========================================================
        s_sb = const.tile([P, RJ], F32)
        nc.sync.dma_start(s_sb[:], alpha_blk.ap().rearrange("(j p) -> p j", p=P))
        nc.scalar.activation(s_sb[:], s_sb[:], AF.Sigmoid)
        nc.vector.tensor_scalar_mul(s_sb[:], s_sb[:], 0.05)

        d_sb = const.tile([P, DKC], F32)
        nc.sync.dma_start(d_sb[:], d_full.ap().rearrange("(q p) -> p q", p=P))
        nc.vector.tensor_scalar(d_sb[:], d_sb[:], 0.0, 1.0, AL.max, AL.min)

        x_lc = recp.tile([P, RJ, D], F32, name="x_lc")
        nc.sync.dma_start(x_lc[:], x_locd.ap().rearrange("(j p) n -> p j n", p=P))

        # SBUF pool stack (LIFO): pg (G combos, lives until F3) ->
        # rtmp (R series, until AG#3) -> ph_ax (X build+powers, until combos)
        pg_st = ExitStack()
        pg = pg_st.enter_context(tc.tile_pool(name="ph_g", bufs=1))
        rt_st = ExitStack()
        rtmp = rt_st.enter_context(tc.tile_pool(name="rtmp", bufs=1))
        pax_st = ExitStack()
        pax = pax_st.enter_context(tc.tile_pool(name="ph_ax", bufs=1))

        # =========================================================
        # Phase A: X rows (bf16, streamed in half-row chunks) -> ccin
        # tiles + local transposes into xt_b; AllGather X      [AG#1]
        # =========================================================
        HD = N // 2
        HT = NKC // 2
        xt_b = pax.tile([P, NKC, RB], BF16, name="xt_b")
        ccin_x = dram.tile([RJ * NKC * P, P], BF16, name="ccin_x")
        paxx_st = ExitStack()
        paxx = paxx_st.enter_context(tc.tile_pool(name="ph_axx", bufs=1))
        for j in range(RJ):
            for h in range(2):
                adj_sb = paxx.tile([P, HD], F32, tag="a_in", bufs=2,
                                   name="adj_sb")
                nc.sync.dma_start(adj_sb[:],
                                  adj_rows[j * P:(j + 1) * P,
                                           h * HD:(h + 1) * HD])
                xbh = paxx.tile([P, HD], BF16, tag="a_b", bufs=2, name="xbh")
                nc.vector.tensor_scalar_mul(xbh[:], adj_sb[:], s_sb[:, j:j + 1])
                put_tiles(ccin_x[(j * NKC + h * HT) * P:
                                 (j * NKC + (h + 1) * HT) * P, :], xbh, HT)
                for tt in range(HT):
                    pe_t(xt_b[:, h * HT + tt, j * P:(j + 1) * P],
                         xbh[:, tt * P:(tt + 1) * P])
        paxx_st.close()
        xfull_g = dram.tile([N_CORES * RJ * NKC * P, P], BF16,
                            addr_space="Shared", name="full_x")
        nc.gpsimd.collective_compute(
            "AllGather", AL.bypass, replica_groups=LGROUP,
            ins=[ccin_x.opt()], outs=[xfull_g.opt()])

        # =========================================================
        # R1: wmat col block = (w diag(d)) @ wT[:, ccols]  (fp32r, local)
        # then Xr col, Xr row block -> AllGather Xr (bf16)      [AG#2]
        # =========================================================
        # fp32r matmul operands must be produced by a rounding op (vector
        # copy/scale into an F32R tile), not raw DMA
        vr_ld = rtmp.tile([P, DKC, FBR], F32, tag="wslab", bufs=1,
                          name="vr_ld")
        nc.sync.dma_start(vr_ld[:],
                          wt_cols.ap().rearrange("(k p) n -> p k n", p=P))
        vr_sb = rtmp.tile([P, DKC, FBR], F32R, name="vr_sb")
        nc.vector.tensor_copy(vr_sb[:], vr_ld[:])
        xr_col = rtmp.tile([P, DKC, FBR], F32, name="xr_col")
        xr_b = rtmp.tile([P, DKC, FBR], BF16, name="xr_b")
        xrrow_b = rtmp.tile([P, D], BF16, name="xrrow_b")
        ccin_xr = dram.tile([DKC * P, P], BF16, name="ccin_xr")
        for m in range(DKC):
            wsl = rtmp.tile([P, DKC, FBR], F32, tag="wslab", bufs=1,
                            name="wslab")
            nc.sync.dma_start(
                wsl[:],
                wT_full[:, m * P:(m + 1) * P].rearrange("(k p) n -> p k n", p=P))
            wsr = rtmp.tile([P, DKC, FBR], F32R, tag="wsr", bufs=1, name="wsr")
            for k in range(DKC):
                nc.vector.tensor_scalar_mul(wsr[:, k, :], wsl[:, k, :],
                                            d_sb[:, k:k + 1])
            ps = psf.tile([P, FBR], F32, tag="mmR", bufs=2, name="ps_mmR")
            for k in range(DKC):
                nc.tensor.matmul(ps[:], wsr[:, k, :], vr_sb[:, k, :],
                                 start=(k == 0), stop=(k == DKC - 1))
            eyef = scrp.tile([P, FBR], F32, tag="eyef", bufs=2, name="eyef")
            nc.sync.dma_start(eyef[:], eye_feat[m * P:(m + 1) * P, :])
            nc.vector.tensor_sub(xr_col[:, m, :], ps[:], eyef[:])
            nc.vector.tensor_scalar_mul(xr_col[:, m, :], xr_col[:, m, :], 0.1)
            nc.vector.tensor_copy(xr_b[:, m, :], xr_col[:, m, :])
            pe_t(xrrow_b[:, m * P:(m + 1) * P], xr_col[:, m, :])
        put_tiles(ccin_xr, xrrow_b, DKC)
        xr_g = dram.tile([N_CORES * DKC * P, P], BF16, addr_space="Shared",
                         name="full_xr")
        nc.gpsimd.collective_compute(
            "AllGather", AL.bypass, replica_groups=LGROUP,
            ins=[ccin_xr.opt()], outs=[xr_g.opt()])

        # ---- pass helpers ----
        def mm_passL(rhs_b, evict):
            for m in range(NKC):
                sl = slabp.tile([P, N_CORES, RJ, P], BF16, tag="xslab", bufs=2,
                                name="xslab")
                nc.sync.dma_start(sl[:], tiled_src(xfull_g, m, RJ, NKC))
                ps = psf.tile([P, RB], F32, tag="mmL", bufs=2, name="ps_mmL")
                for k in range(NKC):
                    nc.tensor.matmul(ps[:], sl[:, k // RJ, k % RJ, :],
                                     rhs_b[:, k, :],
                                     start=(k == 0), stop=(k == NKC - 1))
                evict(m, ps)

        def mm_passR(rhs_b, evict):
            for m in range(DKC):
                sl = slabp.tile([P, N_CORES, 1, P], BF16, tag="rslab", bufs=2,
                                name="rslab")
                nc.sync.dma_start(sl[:], tiled_src(xr_g, m, 1, DKC))
                ps = psf.tile([P, FBR], F32, tag="mmR", bufs=2, name="ps_mmR")
                for k in range(DKC):
                    nc.tensor.matmul(ps[:], sl[:, k, 0, :], rhs_b[:, k, :],
                                     start=(k == 0), stop=(k == DKC - 1))
                evict(m, ps)

        # =========================================================
        # X^2 pass; R powers Xr^2..Xr^5; X^3 pass; combos; ER/ER3
        # =========================================================
        x2t_b = pax.tile([P, NKC, RB], BF16, name="x2t_b")
        mm_passL(xt_b, lambda m, ps: nc.vector.tensor_copy(x2t_b[:, m, :],
                                                           ps[:]))

        xr2_b = rtmp.tile([P, DKC, FBR], BF16, name="xr2_b")
        mm_passR(xr_b, lambda m, ps: nc.vector.tensor_copy(xr2_b[:, m, :],
                                                           ps[:]))
        xr3_b = rtmp.tile([P, DKC, FBR], BF16, name="xr3_b")
        mm_passR(xr2_b, lambda m, ps: nc.vector.tensor_copy(xr3_b[:, m, :],
                                                            ps[:]))

        # X^3 pass: the four L combos consume X^3 straight from PSUM at
        # evict (saves materializing x3t_b)
        gtd_b = [pg.tile([P, NKC, RB], BF16, name=f"g{t}d_b") for t in range(3)]
        elt3_b = lser.tile([P, NKC, RB], BF16, name="elt3_b")
        L_COMBOS = list(zip([*gtd_b, elt3_b], [*GC, EL3C]))

        def ev_x3(m, ps):
            for dst, (c1, c2, c3) in L_COMBOS:
                sc = scrp.tile([P, RB], F32, tag="combo", bufs=2,
                               name="combo_scr")
                nc.vector.tensor_scalar_mul(sc[:], xt_b[:, m, :], c1)
                nc.vector.scalar_tensor_tensor(sc[:], x2t_b[:, m, :], c2,
                                               sc[:], AL.mult, AL.add)
                nc.vector.scalar_tensor_tensor(dst[:, m, :], ps[:], c3, sc[:],
                                               AL.mult, AL.add)
        mm_passL(x2t_b, ev_x3)

        xr4_b = rtmp.tile([P, DKC, FBR], BF16, name="xr4_b")
        mm_passR(xr3_b, lambda m, ps: nc.vector.tensor_copy(xr4_b[:, m, :],
                                                            ps[:]))
        xr5_b = rtmp.tile([P, DKC, FBR], BF16, name="xr5_b")
        mm_passR(xr4_b, lambda m, ps: nc.vector.tensor_copy(xr5_b[:, m, :],
                                                            ps[:]))

        pax_st.close()

        # ---- R combos + ER / ER3 passes ----
        er_rhs = rtmp.tile([P, DKC, FBR], BF16, name="er_rhs")
        er3_rhs = rtmp.tile([P, DKC, FBR], BF16, name="er3_rhs")
        for m in range(DKC):
            sc = scrp.tile([P, FBR], F32, tag="rcombo", bufs=2, name="rcombo")
            nc.vector.tensor_scalar_mul(sc[:], xr2_b[:, m, :], 1.0 / 6.0)
            nc.vector.scalar_tensor_tensor(er_rhs[:, m, :], xr3_b[:, m, :],
                                           1.0 / 24.0, sc[:], AL.mult, AL.add)
            sc2 = scrp.tile([P, FBR], F32, tag="rcombo", bufs=2, name="rcombo2")
            nc.vector.tensor_scalar_mul(sc2[:], xr3_b[:, m, :], ER3C[3])
            nc.vector.scalar_tensor_tensor(sc2[:], xr4_b[:, m, :], ER3C[4],
                                           sc2[:], AL.mult, AL.add)
            nc.vector.scalar_tensor_tensor(er3_rhs[:, m, :], xr5_b[:, m, :],
                                           ER3C[5], sc2[:], AL.mult, AL.add)

        # er / er3 row blocks -> combined row-major AllGather   [AG#3]
        errow_b = rtmp.tile([P, D], BF16, name="errow_b")
        er3row_b = rtmp.tile([P, D], BF16, name="er3row_b")
        ccin_er = dram.tile([P, 2 * D], BF16, name="ccin_er")

        def ev_er(m, ps):
            sc = scrp.tile([P, FBR], F32, tag="erc", bufs=2, name="er_scr")
            nc.vector.scalar_tensor_tensor(sc[:], xr2_b[:, m, :], 0.5,
                                           xr_col[:, m, :], AL.mult, AL.add)
            nc.vector.tensor_add(sc[:], sc[:], ps[:])
            pe_t(errow_b[:, m * P:(m + 1) * P], sc[:])
        mm_passR(er_rhs, ev_er)

        def ev_er3(m, ps):
            sc = scrp.tile([P, FBR], F32, tag="erc", bufs=2, name="er3_scr")
            nc.vector.tensor_scalar_mul(sc[:], xr_col[:, m, :], ER3C[0])
            nc.vector.scalar_tensor_tensor(sc[:], xr2_b[:, m, :], ER3C[1],
                                           sc[:], AL.mult, AL.add)
            nc.vector.scalar_tensor_tensor(sc[:], xr3_b[:, m, :], ER3C[2],
                                           sc[:], AL.mult, AL.add)
            nc.vector.tensor_add(sc[:], sc[:], ps[:])
            pe_t(er3row_b[:, m * P:(m + 1) * P], sc[:])
        mm_passR(er3_rhs, ev_er3)

        nc.sync.dma_start(ccin_er[:, 0:D], errow_b[:])
        nc.sync.dma_start(ccin_er[:, D:2 * D], er3row_b[:])
        er_g = dram.tile([N_CORES * P, 2 * D], BF16, addr_space="Shared",
                         name="full_er")
        nc.gpsimd.collective_compute(
            "AllGather", AL.bypass, replica_groups=LGROUP,
            ins=[ccin_er.opt()], outs=[er_g.opt()])

        pf_st.close()
        rt_st.close()
        psr = top.enter_context(tc.tile_pool(name="psumr", bufs=1,
                                             space="PSUM"))

        # =========================================================
        # P passes:  P_t = 0.1*x0_loc + G_tD @ x0   (row form, local;
        # x0 chunks streamed from DRAM, all 3 t's share each chunk)
        # =========================================================
        pp_st = ExitStack()
        pp = pp_st.enter_context(tc.tile_pool(name="ph_p", bufs=1))
        x0_lc = pp.tile([P, RJ, D], F32, name="x0_lc")
        nc.sync.dma_start(x0_lc[:],
                          x0_locd.ap().rearrange("(j p) n -> p j n", p=P))
        pt = [pp.tile([P, RJ, D], F32, name=f"p{t}") for t in range(3)]
        for j in range(RJ):
            pss = [psr.tile([P, FH], F32, tag=f"f{i}", bufs=1,
                            name=f"ps_f{i}") for i in range(6)]
            for k in range(NKC):
                ld = slabp.tile([P, D], F32, tag="ld", bufs=2, name="ld_scr")
                nc.sync.dma_start(ld[:], x0_full[k * P:(k + 1) * P, :])
                cb = slabp.tile([P, D], BF16, tag="icc", bufs=2, name="cb")
                nc.vector.tensor_copy(cb[:], ld[:])
                for t in range(3):
                    for f in range(2):
                        nc.tensor.matmul(pss[2 * t + f][:],
                                         gtd_b[t][:, k, j * P:(j + 1) * P],
                                         cb[:, f * FH:(f + 1) * FH],
                                         start=(k == 0), stop=(k == NKC - 1))
            for t in range(3):
                for f in range(2):
                    nc.vector.scalar_tensor_tensor(
                        pt[t][:, j, f * FH:(f + 1) * FH],
                        x0_lc[:, j, f * FH:(f + 1) * FH], 0.1,
                        pss[2 * t + f][:], AL.mult, AL.add)

        # er/er3 rows into SBUF: er_rows[:,k,:] = ER[kblk, :]
        er_rows = rser.tile([P, DKC, D], BF16, name="er_rows")
        er3_rows = rser.tile([P, DKC, D], BF16, name="er3_rows")
        for k in range(DKC):
            nc.sync.dma_start(er_rows[:, k, :], er_g[k * P:(k + 1) * P, 0:D])
            nc.sync.dma_start(er3_rows[:, k, :],
                              er_g[k * P:(k + 1) * P, D:2 * D])

        # ---- W-type pass: out(j,f) = sum_k Z^T[k,j].T @ er[k, fslice] ----
        def w_pass(zb_rows, er_sb, evict):
            zt = recp.tile([P, DKC, RB], BF16, tag="zt", bufs=1, name="zt_b")
            for j in range(RJ):
                for k in range(DKC):
                    pe_t(zt[:, k, j * P:(j + 1) * P],
                         zb_rows[:, j, k * P:(k + 1) * P])
            for j in range(RJ):
                pss = [psr.tile([P, FH], F32, tag=f"f{f}", bufs=1,
                                name=f"ps_f{f}") for f in range(2)]
                for k in range(DKC):
                    for f in range(2):
                        nc.tensor.matmul(pss[f][:], zt[:, k, j * P:(j + 1) * P],
                                         er_sb[:, k, f * FH:(f + 1) * FH],
                                         start=(k == 0), stop=(k == DKC - 1))
                for f in range(2):
                    evict(j, f, pss[f])

        # ---- F3 = P0 + (P1 + P2@R)@R ----
        q = pp.tile([P, RJ, D], F32, name="q_rows")
        qb = recp.tile([P, RJ, D], BF16, tag="qb", bufs=1, name="qb")
        nc.vector.tensor_copy(qb[:], pt[2][:])
        w_pass(qb, er_rows,
               lambda j, f, ps: nc.vector.tensor_add(
                   q[:, j, f * FH:(f + 1) * FH], ps[:],
                   pt[2][:, j, f * FH:(f + 1) * FH]))
        nc.vector.tensor_add(q[:], q[:], pt[1][:])
        qb2 = recp.tile([P, RJ, D], BF16, tag="qb", bufs=1, name="qb2")
        nc.vector.tensor_copy(qb2[:], q[:])
        f3 = recp.tile([P, RJ, D], F32, name="f3_rows")

        def ev_f3(j, f, ps):
            sl = (slice(None), j, slice(f * FH, (f + 1) * FH))
            nc.vector.tensor_add(f3[sl], ps[:], q[sl])
            nc.vector.tensor_add(f3[sl], f3[sl], pt[0][sl])
        w_pass(qb2, er_rows, ev_f3)
        pp_st.close()
        pg_st.close()

        # =========================================================
        # 3 recurrence steps: IC' = S + S@ER3 + F3,  S = IC + EL3@IC
        # =========================================================
        ic_g = [None, None]
        ic_state = [None]

        def step_ic(t):
            # V pass, k-outer: each rhs chunk DMA'd once, feeds 4 psums
            pss = [psr.tile([P, FH], F32, tag=f"f{i}", bufs=1,
                            name=f"ps_f{i}") for i in range(4)]
            for k in range(NKC):
                if t == 0:
                    ld = slabp.tile([P, D], F32, tag="ld", bufs=2,
                                    name="ld_scr")
                    nc.sync.dma_start(ld[:], x_full[k * P:(k + 1) * P, :])
                    rk = slabp.tile([P, D], BF16, tag="icc", bufs=2, name="cb")
                    nc.vector.tensor_copy(rk[:], ld[:])
                else:
                    rk = slabp.tile([P, D], BF16, tag="icc", bufs=2, name="cb")
                    nc.sync.dma_start(rk[:], ic_g[t - 1][k * P:(k + 1) * P, :])
                for j in range(RJ):
                    for f in range(2):
                        nc.tensor.matmul(pss[2 * j + f][:],
                                         elt3_b[:, k, j * P:(j + 1) * P],
                                         rk[:, f * FH:(f + 1) * FH],
                                         start=(k == 0), stop=(k == NKC - 1))
            src = x_lc if t == 0 else ic_state[0]
            s_rows = recp.tile([P, RJ, D], F32, tag="s", bufs=1, name="s_rows")
            for j in range(RJ):
                for f in range(2):
                    sl = (slice(None), j, slice(f * FH, (f + 1) * FH))
                    nc.vector.tensor_add(s_rows[sl], pss[2 * j + f][:], src[sl])
            sb = recp.tile([P, RJ, D], BF16, tag="qb", bufs=1, name="sb")
            nc.vector.tensor_copy(sb[:], s_rows[:])

            out = recp.tile([P, RJ, D], F32, tag="ic", bufs=2,
                            name="ic_rows" if t < 2 else "z_rows")

            def ev(j, f, ps):
                sl = (slice(None), j, slice(f * FH, (f + 1) * FH))
                nc.vector.tensor_add(out[sl], ps[:], s_rows[sl])
                nc.vector.tensor_add(out[sl], out[sl], f3[sl])
            w_pass(sb, er3_rows, ev)

            if t < 2:
                ic_state[0] = out
                ob = recp.tile([P, RJ, D], BF16, tag="qb", bufs=1, name="ob")
                nc.vector.tensor_copy(ob[:], out[:])
                ccin = dram.tile([RB, D], BF16, tag="ccin_ic",
                                 name=f"ccin_ic{t}")
                for j in range(RJ):
                    nc.sync.dma_start(ccin[j * P:(j + 1) * P, :], ob[:, j, :])
                g = dram.tile([N, D], BF16, addr_space="Shared",
                              name=f"full_ic{t}")
                nc.gpsimd.collective_compute(
                    "AllGather", AL.bypass, replica_groups=LGROUP,
                    ins=[ccin.opt()], outs=[g.opt()])
                ic_g[t] = g
            else:
                for j in range(RJ):
                    nc.sync.dma_start(z_loc[j * P:(j + 1) * P, :], out[:, j, :])

        for t in range(3):
            step_ic(t)

    nc.compile()
    return nc


_NC_CACHE = []


def _get_nc():
    if not _NC_CACHE:
        _NC_CACHE.append(build_nc())
    return _NC_CACHE[0]


def make_in_maps(inputs):
    x = np.ascontiguousarray(np.asarray(inputs["x"], dtype=np.float32))
    x0 = np.ascontiguousarray(np.asarray(inputs["x0"], dtype=np.float32))
    adj = np.ascontiguousarray(np.asarray(inputs["adj"], dtype=np.float32))
    alpha = np.ascontiguousarray(
        np.asarray(inputs["alpha_train"], dtype=np.float32))
    w = np.ascontiguousarray(np.asarray(inputs["w"], dtype=np.float32))
    d = np.ascontiguousarray(np.asarray(inputs["d"], dtype=np.float32))

    eye_n = np.eye(N, dtype=np.float32)
    eye_d = np.eye(D, dtype=np.float32)
    wT = np.ascontiguousarray(w.T)

    in_maps = []
    for c in range(N_CORES):
        r0 = c * RB
        f0 = c * FBR
        in_maps.append({
            "adj_rows": np.ascontiguousarray(
                adj[r0:r0 + RB, :] - eye_n[r0:r0 + RB, :]),
            "alpha_blk": np.ascontiguousarray(alpha[r0:r0 + RB]),
            "x_full": x,
            "x0_full": x0,
            "x_loc": np.ascontiguousarray(x[r0:r0 + RB, :]),
            "x0_loc": np.ascontiguousarray(x0[r0:r0 + RB, :]),
            "wT_full": wT,
            "wt_cols": np.ascontiguousarray(wT[:, f0:f0 + FBR]),
            "eye_feat": np.ascontiguousarray(eye_d[:, f0:f0 + FBR]),
            "d_full": d,
        })
    return in_maps


def kernel(**inputs) -> np.ndarray:
    nc = _get_nc()
    in_maps = make_in_maps(inputs)
    res = run_bass_kernel_spmd(nc, in_maps, core_ids=list(range(N_CORES)))
    z = np.concatenate([res.results[c]["z_loc"] for c in range(N_CORES)],
                       axis=0)
    return np.ascontiguousarray(z.astype(np.float32))


if __name__ == "__main__":
    rng = np.random.default_rng(0)
    ins = {
        "x": rng.standard_normal((N, D)).astype(np.float32),
        "x0": rng.standard_normal((N, D)).astype(np.float32),
        "adj": (rng.random((N, N)) / N).astype(np.float32),
        "alpha_train": rng.standard_normal((N,)).astype(np.float32),
        "w": (np.eye(D) + 0.02 * rng.standard_normal((D, D))).astype(np.float32),
        "d": rng.random((D,)).astype(np.float32),
    }
    out = kernel(**ins)
    print("kernel output:", out.shape, out.dtype, float(np.linalg.norm(out)))
